# revision 24
# baseline (speedup 1.0000x reference)
"""Trainium2 Bass kernel for nn_ECPEC (emotion-cause pair extraction).

Sharding: data-parallel over the 256 emotion utterances j -> 32 rows per
NeuronCore; cause-side encodings (rep layer, GRU chunk encoder) and all
weights are replicated on every core.  The device computes DENSE outputs
over all (j, k) chunk pairs; the ragged teacher-forcing selection
(nonzeros of label_ck) and label3_out are pure host-side indexing, so the
compiled program is input-shape independent.

Math restructuring vs. the reference graph (all matmuls in float32r):
  phase2:  sigmoid([emo, ch] @ Ww.T + b) == sigmoid(W1@emo.T + W2@ch.T + b)
  phase3:  delta @ W3.T == We@e_j + Wt@c_s + wn*||e_j-c_s|| + Wet@(e_j*c_s) + b3
           with ||e-c||^2 == e2 - 2 e.c + c2 computed via Gram matmuls.
           Wt@c / wn-outer-product / bias terms are accumulated into the
           same PSUM group via identity / rank-1 matmuls; We@e_j + b3 is
           the per-partition bias of the Relu activation.
  log_softmax over 2 classes: d = x1 - x0;
           out = (-softplus(d), d - softplus(d)) via ScalarE Softplus.
"""

from contextlib import ExitStack

import numpy as np

import concourse.bass as bass
import concourse.tile as tile
from concourse import bacc, mybir
from concourse.bass_utils import run_bass_kernel_spmd

D = 512
SE = 256
SC = 512
CS = 8
K = 64              # number of cause chunks
NCORES = 8
JC = SE // NCORES   # 32 emotion rows per core
COLS = JC * SC      # 16384 dense phase3 rows per core ((j,k,t) = (j,s))
P2 = JC * K         # 2048 dense phase2 rows per core

FR = mybir.dt.float32r
F32 = mybir.dt.float32
AF = mybir.ActivationFunctionType
OP = mybir.AluOpType
ts = bass.ts
ds = bass.ds


def _ap(base, ap_list, extra_offset=0):
    return bass.AP(tensor=base.tensor, offset=base.offset + extra_offset,
                   ap=[list(x) for x in ap_list])


def build_nc():
    nc = bacc.Bacc("TRN2", target_bir_lowering=False, debug=False,
                   num_devices=NCORES)

    names_fr = {
        "temoT": [128, 4 * JC], "tcauT": [128, 4 * SC],
        "repT": [128, 4 * D], "w1T": [128, 4 * D], "w2T": [128, 4 * D],
        "wb_row": [1, D], "wod": [128, 4],
        "wihT": [128, 4 * 3 * D], "whhT": [128, 4 * 3 * D],
        "grub_row": [1, 3 * D], "bhn_row": [1, D],
        "weT": [128, 4 * D], "wtT": [128, 4 * D], "wetT": [128, 4 * D],
        "wn_row": [1, D], "b3_row": [1, D], "clsd": [128, 4],
        "ones_row": [1, 128], "ones_blk": [128, JC], "id128": [128, 128],
    }
    dram = {}
    for n, shp in names_fr.items():
        dram[n] = nc.dram_tensor(n, shp, FR, kind="ExternalInput")
    for n in ("rep_b", "wobb", "clsbb", "wobbn", "clsbbn"):
        dram[n] = nc.dram_tensor(n, [128, 4] if n == "rep_b" else [128, 1],
                                 F32, kind="ExternalInput")
    dram["p2out"] = nc.dram_tensor("p2out", [P2, 2], F32, kind="ExternalOutput")
    dram["p3out"] = nc.dram_tensor("p3out", [COLS, 2], F32,
                                   kind="ExternalOutput")

    with tile.TileContext(nc, pool_alloc_mode="queue") as tc:
        _emit(tc, dram)
    nc.compile()
    return nc


def _emit(tc, dram):
    nc = tc.nc
    ctx = ExitStack()
    with ctx:
        res = ctx.enter_context(tc.tile_pool(name="res", bufs=1))
        ps_z = ctx.enter_context(tc.tile_pool(name="ps_z", bufs=3,
                                              space="PSUM"))
        ps_gru = ctx.enter_context(tc.tile_pool(name="ps_gru", bufs=1,
                                                space="PSUM"))
        ps_d = ctx.enter_context(tc.tile_pool(name="ps_d", bufs=1,
                                              space="PSUM"))

        gxp = ctx.enter_context(tc.tile_pool(name="gxp", bufs=1))
        gx3 = gxp.tile([K, CS, 3 * D], FR, tag="gx3")
        w0_cm = tc.tile_pool(name="w0", bufs=1)
        w0 = w0_cm.__enter__()

        def load(pool, name, shape, dt=FR, eng=None):
            tl = pool.tile(shape, dt, tag=name)
            (eng or nc.sync).dma_start(tl[:], dram[name].ap())
            return tl

        # critical-path loads first (idn feeds the PE warm-up spin)
        idn = load(res, "id128", [128, 128])
        rep = load(w0, "repT", [128, 4, D])
        repb = load(w0, "rep_b", [128, 4], F32)
        tem = load(w0, "temoT", [128, 4, JC])
        tca = load(w0, "tcauT", [128, 4, SC])
        wtw = load(w0, "wtT", [128, 4, D])
        wih = load(w0, "wihT", [128, 4, 3 * D])
        we = load(w0, "weT", [128, 4, D])
        w1 = load(w0, "w1T", [128, 4, D])
        wet = load(res, "wetT", [128, 4, D], eng=nc.scalar)

        # resident tensors
        whh = load(res, "whhT", [128, 4, 3 * D], eng=nc.scalar)
        w2 = load(res, "w2T", [128, 4, D], eng=nc.scalar)
        wnr = load(res, "wn_row", [1, D])
        bhnr = load(res, "bhn_row", [1, D])
        onesr = load(res, "ones_row", [1, 128])
        onesb = load(res, "ones_blk", [128, JC])
        clsd_s = load(res, "clsd", [128, 4])
        wod_s = load(res, "wod", [128, 4])
        wobb = load(res, "wobb", [128, 1], F32)
        clsbb = load(res, "clsbb", [128, 1], F32)
        wobbn = load(res, "wobbn", [128, 1], F32)
        clsbbn = load(res, "clsbbn", [128, 1], F32)

        eT = res.tile([128, 4, JC], FR, tag="eT")
        cT = res.tile([128, 4, SC], FR, tag="cT")
        cauW = res.tile([128, 4, SC], FR, tag="cauW")
        ejb = res.tile([128, 4, JC], F32, tag="ejb")
        atb = res.tile([128, 4, JC], F32, tag="atb")
        btk = res.tile([128, 4, K], F32, tag="btk")
        e2T = res.tile([JC, 1], F32, tag="e2T")
        nrm32 = res.tile([JC, SC], FR, tag="nrm32")

        # remaining one-shot weights
        wbr = load(w0, "wb_row", [1, D])
        b3r = load(w0, "b3_row", [1, D])
        grubr = load(w0, "grub_row", [1, 3 * D])



        # ================= setup =========================================
        # PE warm-up spin: keep TensorE busy through the DMA-in phase so the
        # HAM clock gate opens before real work starts (write-only psum).
        warm = ps_z.tile([128, 128], F32, tag="z")
        for _ in range(72):
            nc.tensor.matmul(warm[:], idn[:, :], idn[:, :],
                             start=True, stop=True)

        # rep layer: e = relu(W@emo.T + b), c = relu(W@cau.T + b)
        for m in range(4):
            pp = ps_z.tile([128, JC], F32, tag="z")
            for k in range(4):
                nc.tensor.matmul(pp[:], rep[:, k, ts(m, 128)],
                                 tem[:, k, :], start=(k == 0), stop=(k == 3))
            nc.scalar.activation(eT[:, m, :], pp[:], AF.Relu,
                                 bias=repb[:, m:m + 1], scale=1.0)
        for m in range(4):
            pp = ps_z.tile([128, SC], F32, tag="z")
            for k in range(4):
                nc.tensor.matmul(pp[:], rep[:, k, ts(m, 128)],
                                 tca[:, k, :], start=(k == 0), stop=(k == 3))
            nc.scalar.activation(cT[:, m, :], pp[:], AF.Relu,
                                 bias=repb[:, m:m + 1], scale=1.0)

        # cauW = Wt @ cau
        for m in range(4):
            pp = ps_z.tile([128, SC], F32, tag="z")
            for k in range(4):
                nc.tensor.matmul(pp[:], wtw[:, k, ts(m, 128)],
                                 cT[:, k, :], start=(k == 0), stop=(k == 3))
            nc.vector.tensor_copy(cauW[:, m, :], pp[:])

        # ||e_j - c_s||: via e2 - 2 e.c + c2 (DVE runs under the PE setup)
        sq_c = w0.tile([128, 4, SC], FR, tag="sqc")
        nc.vector.tensor_mul(sq_c[:], cT[:], cT[:])
        sq_e = w0.tile([128, 4, JC], FR, tag="sqe")
        nc.vector.tensor_mul(sq_e[:], eT[:], eT[:])
        em2 = w0.tile([128, 4, JC], FR, tag="em2")
        nc.vector.tensor_scalar_mul(em2[:], eT[:], -2.0)

        e2ps = ps_d.tile([1, JC], F32, tag="d")
        for k in range(4):
            nc.tensor.matmul(e2ps[:], onesb[:, 0:1], sq_e[:, k, :],
                             start=(k == 0), stop=(k == 3))
        e2sb = w0.tile([1, JC], F32, tag="e2sb")
        nc.vector.tensor_copy(e2sb[:], e2ps[:])
        nc.sync.dma_start(e2T[:], e2sb[:])          # [1,32] -> [32,1]

        dot = ps_z.tile([JC, SC], F32, tag="z")
        for k in range(4):
            nc.tensor.matmul(dot[:], em2[:, k, :], cT[:, k, :],
                             start=(k == 0), stop=False)
        for k in range(4):
            nc.tensor.matmul(dot[:], onesb[:, :], sq_c[:, k, :],
                             start=False, stop=(k == 3))
        nsq = w0.tile([JC, SC], F32, tag="nsq")
        nc.vector.tensor_scalar(nsq[:], dot[:], e2T[:], 0.0, OP.add, OP.max)
        nc.scalar.activation(nrm32[:], nsq[:], AF.Sqrt)

        def gx_block(m):
            # GX rows 128m..128m+128 of cau @ wih.T + grub -> gx3[k, t, 3D].
            # Even m: one 3-bank psum group (gh slot); odd m: 3 z-slot
            # groups.  Alternation means a slot is never reused until two
            # groups later, so the ScalarE drain latency stays hidden.
            if m % 2 == 0:
                pp = ps_gru.tile([128, 3 * D], F32, tag="gh")
                for g in range(3):
                    for k in range(4):
                        nc.tensor.matmul(pp[:, ds(g * D, D)],
                                         cT[:, k, ts(m, 128)],
                                         wih[:, k, ds(g * D, D)],
                                         start=(k == 0), stop=False)
                    nc.tensor.matmul(pp[:, ds(g * D, D)], onesr[0:1, 0:128],
                                     grubr[0:1, ds(g * D, D)],
                                     start=False, stop=True)
                stg = w0.tile([128, 3 * D], FR, tag="gxstg")
                nc.scalar.activation(stg[:], pp[:], AF.Copy)
                nc.sync.dma_start(gx3[16 * m:16 * (m + 1), :, :], stg[:])
            else:
                for g in range(3):
                    pp = ps_z.tile([128, D], F32, tag="z")
                    for k in range(4):
                        nc.tensor.matmul(pp[:], cT[:, k, ts(m, 128)],
                                         wih[:, k, ds(g * D, D)],
                                         start=(k == 0), stop=False)
                    nc.tensor.matmul(pp[:], onesr[0:1, 0:128],
                                     grubr[0:1, ds(g * D, D)],
                                     start=False, stop=True)
                    stg2 = w0.tile([128, D], FR, tag="gxstg2")
                    nc.scalar.activation(stg2[:], pp[:], AF.Copy)
                    nc.sync.dma_start(
                        gx3[16 * m:16 * (m + 1), :, ds(g * D, D)], stg2[:])

        # re-warm the PE (DMA waits may have idled it past the HAM window)
        warm2 = ps_z.tile([128, 128], F32, tag="z")
        for _ in range(24):
            nc.tensor.matmul(warm2[:], idn[:, :], idn[:, :],
                             start=True, stop=True)
        def ejb_block(m):
            pp = ps_z.tile([128, JC], F32, tag="z")
            for k in range(4):
                nc.tensor.matmul(pp[:], we[:, k, ts(m, 128)],
                                 eT[:, k, :], start=(k == 0), stop=False)
            nc.tensor.matmul(pp[:], b3r[0:1, ts(m, 128)],
                             onesr[0:1, 0:JC], start=False, stop=True)
            nc.vector.tensor_copy(ejb[:, m, :], pp[:])

        def atb_block(m):
            pp = ps_z.tile([128, JC], F32, tag="z")
            for k in range(4):
                nc.tensor.matmul(pp[:], w1[:, k, ts(m, 128)],
                                 eT[:, k, :], start=(k == 0), stop=False)
            nc.tensor.matmul(pp[:], wbr[0:1, ts(m, 128)],
                             onesr[0:1, 0:JC], start=False, stop=True)
            nc.vector.tensor_copy(atb[:, m, :], pp[:])

        gx_block(0)
        for m in range(4):
            ejb_block(m)
        gx_block(1)
        for m in range(4):
            atb_block(m)
        gx_block(2)
        gx_block(3)


        w0_cm.__exit__(None, None, None)

        # ============ main loop: phase3 over j, GRU + phase2 woven in ====
        scr1a = ctx.enter_context(tc.tile_pool(name="scr1a", bufs=1))
        scr1b = ctx.enter_context(tc.tile_pool(name="scr1b", bufs=2))
        scr2 = ctx.enter_context(tc.tile_pool(name="scr2", bufs=2))
        scr3 = ctx.enter_context(tc.tile_pool(name="scr3", bufs=1))
        nrmp = ctx.enter_context(tc.tile_pool(name="nrmp", bufs=4))
        fin = ctx.enter_context(tc.tile_pool(name="fin", bufs=1))
        sd2 = fin.tile([4, D], F32, tag="sd2")
        sd3 = fin.tile([JC, SC], F32, tag="sd3")

        hrow = scr1b.tile([K, D], F32, tag="hrow")
        nc.vector.memset(hrow[:], 0.0)
        hT = None

        def gru_step(st):
            nonlocal hrow, hT
            gh = ps_gru.tile([K, 3 * D], F32, tag="gh")
            for g in range(3):
                sl = ds(g * D, D)
                if g < 2:
                    nc.tensor.matmul(gh[:, sl], idn[0:K, 0:K],
                                     gx3[:, st, sl], start=True,
                                     stop=(st == 0))
                else:
                    nc.tensor.matmul(gh[:, sl], onesr[0:1, 0:K],
                                     bhnr[0:1, :], start=True, stop=(st == 0))
                if st > 0:
                    for k in range(4):
                        nc.tensor.matmul(gh[:, sl], hT[:, ts(k, K)],
                                         whh[:, k, sl], start=False,
                                         stop=(k == 3))
            rz = scr1a.tile([K, 2 * D], F32, tag="rz")
            nc.scalar.activation(rz[:], gh[:, 0:2 * D], AF.Sigmoid)
            t2 = scr1a.tile([K, D], F32, tag="t2")
            nc.vector.tensor_mul(t2[:], gh[:, 2 * D:3 * D], rz[:, 0:D])
            # SBUF-only steps go to the (otherwise idle) GPSIMD engine so the
            # serial GRU chain does not block the DVE queue feeding phase 3.
            t3 = scr1a.tile([K, D], F32, tag="t3")
            nc.vector.tensor_add(t3[:], t2[:], gx3[:, st, 2 * D:3 * D])
            n_t = scr1a.tile([K, D], F32, tag="n_t")
            nc.scalar.activation(n_t[:], t3[:], AF.Tanh)
            t4 = scr1a.tile([K, D], F32, tag="t4")
            nc.vector.tensor_sub(t4[:], hrow[:], n_t[:])
            t5 = scr1a.tile([K, D], F32, tag="t5")
            nc.vector.tensor_mul(t5[:], rz[:, D:2 * D], t4[:])
            hrow = scr1b.tile([K, D], FR, tag="hrow")
            nc.vector.tensor_add(hrow[:], n_t[:], t5[:])
            tp = ps_gru.tile([128, 4 * K], FR, tag="tp")
            for mc in range(4):
                nc.tensor.transpose(tp[:, ts(mc, K)], hrow[:, ts(mc, 128)],
                                    idn[0:K, 0:K])
            hT = scr1b.tile([128, 4 * K], FR, tag="hT")
            nc.vector.tensor_copy(hT[:], tp[:])

        def btk_calc():
            for m in range(4):
                pp = ps_z.tile([128, K], F32, tag="z")
                for k in range(4):
                    nc.tensor.matmul(pp[:], w2[:, k, ts(m, 128)],
                                     hT[:, ts(k, K)], start=(k == 0),
                                     stop=(k == 3))
                nc.vector.tensor_copy(btk[:, m, :], pp[:])

        p2hid = {}

        def phase2_produce(g):
            hidc = scr3.tile([128, 4, D], FR, tag="hidc")
            for m in range(4):
                tt = scr2.tile([128, D], F32, tag="p2t")
                a0 = atb[:, m, ds(8 * g, 8)]
                b0 = btk[:, m, :]
                a_ap = _ap(a0, [a0.ap[0], a0.ap[1], [0, K]])
                b_ap = _ap(b0, [b0.ap[0], [0, 8], b0.ap[1]])
                nc.vector.tensor_tensor(tt[:], a_ap, b_ap, OP.add)
                nc.scalar.activation(hidc[:, m, :], tt[:], AF.Sigmoid)
            p2hid[g] = hidc

        def phase2_mm(g):
            hidc = p2hid.pop(g)
            dp = ps_d.tile([1, D], F32, tag="d")
            for m in range(4):
                nc.tensor.matmul(dp[:], wod_s[:, m:m + 1], hidc[:, m, :],
                                 start=(m == 0), stop=(m == 3))
            dstg = scr2.tile([1, D], F32, tag="dstg")
            nc.vector.tensor_copy(dstg[:], dp[:])
            nc.sync.dma_start(sd2[g:g + 1, :], dstg[:])

        def fetch_nrm(j):
            nrmj = nrmp.tile([1, SC], FR, tag="nrmj")
            nc.gpsimd.dma_start(nrmj[:], nrm32[j:j + 1, :])
            return nrmj

        def build_xt(j):
            xt = scr2.tile([128, 4, SC], FR, tag="xt")
            for m in range(4):
                nc.vector.tensor_scalar_mul(
                    xt[:, m, :], cT[:, m, :],
                    eT[:, m, j:j + 1].bitcast(F32))
            return xt

        nrm_q = [fetch_nrm(0), fetch_nrm(1), fetch_nrm(2)]
        nxt = build_xt(0)
        for j in range(JC):
            xt = nxt
            nrmj = nrm_q.pop(0)
            if j + 3 < JC:
                nrm_q.append(fetch_nrm(j + 3))
            hj = scr3.tile([128, 4, SC], FR, tag="hj")
            for m in range(4):
                zp = ps_z.tile([128, SC], F32, tag="z")
                for k in range(4):
                    nc.tensor.matmul(zp[:], wet[:, k, ts(m, 128)],
                                     xt[:, k, :], start=(k == 0), stop=False)
                nc.tensor.matmul(zp[:], wnr[0:1, ts(m, 128)],
                                 nrmj[0:1, :], start=False, stop=False)
                nc.tensor.matmul(zp[:], idn[:, :], cauW[:, m, :],
                                 start=False, stop=True)
                nc.scalar.activation(hj[:, m, :], zp[:], AF.Relu,
                                     bias=ejb[:, m, j:j + 1], scale=1.0)

            # next iteration's rhs BEFORE this iteration's d-projection: the
            # dstg psum-drain copy below waits on the d matmuls, and anything
            # queued after it on the in-order DVE stream would stall with it
            # — building xt[j+1] first keeps the PE fed.
            if j + 1 < JC:
                nxt = build_xt(j + 1)

            dp = ps_d.tile([1, SC], F32, tag="d")
            for k in range(4):
                nc.tensor.matmul(dp[:], clsd_s[:, k:k + 1], hj[:, k, :],
                                 start=(k == 0), stop=(k == 3))
            dstg = scr2.tile([1, SC], F32, tag="dstg")
            nc.vector.tensor_copy(dstg[:], dp[:])
            nc.sync.dma_start(sd3[j:j + 1, :], dstg[:])


            # weave the serial GRU + small phase2 blocks into the PE stream
            if j >= 4 and j % 2 == 0 and (j - 4) // 2 < 8:
                gru_step((j - 4) // 2)
                if (j - 4) // 2 == 7:
                    btk_calc()
            if j in (20, 22, 24, 26):
                phase2_produce((j - 20) // 2)
            if j in (22, 24, 26, 28):
                phase2_mm((j - 22) // 2)

        # phase2 log-softmax tail
        sg2 = fin.tile([4, D], F32, tag="sg2")
        nc.scalar.activation(sg2[:], sd2[:], AF.Sigmoid,
                             bias=wobbn[0:4, :], scale=-1.0)
        o2 = fin.tile([4, 2 * D], F32, tag="o2")
        o2e = _ap(o2[:], [o2[:].ap[0], [2, D]])
        nc.scalar.activation(o2e, sg2[:], AF.Ln)
        nc.vector.scalar_tensor_tensor(
            _ap(o2[:], [o2[:].ap[0], [2, D]], extra_offset=1),
            sd2[:], wobb[0:4, :], o2e, OP.add, OP.add)
        nc.sync.dma_start(dram["p2out"].ap(), o2[:])

        # phase3 log-softmax tail
        sg3 = fin.tile([JC, SC], F32, tag="sg3")
        nc.scalar.activation(sg3[:], sd3[:], AF.Sigmoid,
                             bias=clsbbn[0:JC, :], scale=-1.0)
        o3 = fin.tile([JC, 2 * SC], F32, tag="o3")
        o3e = _ap(o3[:], [o3[:].ap[0], [2, SC]])
        nc.scalar.activation(o3e, sg3[:], AF.Ln)
        nc.vector.scalar_tensor_tensor(
            _ap(o3[:], [o3[:].ap[0], [2, SC]], extra_offset=1),
            sd3[:], clsbb[0:JC, :], o3e, OP.add, OP.add)
        nc.sync.dma_start(dram["p3out"].ap(), o3[:])


# ------------------------------------------------------------------ host ----

def _chunk(w):
    """[512, N] -> [128, 4, N] flattened as [128, 4*N] (partition-chunked)."""
    n = w.shape[1]
    return np.ascontiguousarray(
        w.reshape(4, 128, n).transpose(1, 0, 2).reshape(128, 4 * n))


_CACHE = {}
TRACE = False          # test harness sets True to collect NTFF exec time


def kernel(text_emo, text_cau, label_ck, label3, chunksize, t_ratio,
           rep_w, rep_b, W_w, W_b, Wo_w, Wo_b,
           gru_wih, gru_whh, gru_bih, gru_bhh, W3_w, W3_b, cls_w, cls_b):
    text_emo = np.asarray(text_emo, np.float32)
    text_cau = np.asarray(text_cau, np.float32)
    label_ck = np.asarray(label_ck)
    label3 = np.asarray(label3)
    assert int(chunksize) == CS and int(t_ratio) == 1
    f32 = lambda x: np.asarray(x, np.float32)
    rep_w, rep_b, W_w, W_b, Wo_w, Wo_b = map(
        f32, (rep_w, rep_b, W_w, W_b, Wo_w, Wo_b))
    gru_wih, gru_whh, gru_bih, gru_bhh = map(
        f32, (gru_wih, gru_whh, gru_bih, gru_bhh))
    W3_w, W3_b, cls_w, cls_b = map(f32, (W3_w, W3_b, cls_w, cls_b))

    if "nc" not in _CACHE:
        _CACHE["nc"] = build_nc()
    nc = _CACHE["nc"]

    grub = np.concatenate([gru_bih[:2 * D] + gru_bhh[:2 * D],
                           gru_bih[2 * D:]])
    shared = {
        "tcauT": _chunk(text_cau[:, 0, :].T.copy()),
        "repT": _chunk(rep_w.T.copy()),
        "rep_b": np.ascontiguousarray(rep_b.reshape(4, 128).T),
        "w1T": _chunk(W_w[:, :D].T.copy()),
        "w2T": _chunk(W_w[:, D:].T.copy()),
        "wb_row": W_b.reshape(1, D).copy(),
        "wod": np.ascontiguousarray((Wo_w[1] - Wo_w[0]).reshape(4, 128).T),
        "wihT": _chunk(gru_wih.T.copy()),
        "whhT": _chunk(gru_whh.T.copy()),
        "grub_row": grub.reshape(1, 3 * D),
        "bhn_row": gru_bhh[2 * D:].reshape(1, D).copy(),
        "weT": _chunk(W3_w[:, :D].T.copy()),
        "wtT": _chunk(W3_w[:, D:2 * D].T.copy()),
        "wetT": _chunk(W3_w[:, 2 * D + 1:].T.copy()),
        "wn_row": W3_w[:, 2 * D].reshape(1, D).copy(),
        "b3_row": W3_b.reshape(1, D).copy(),
        "clsd": np.ascontiguousarray((cls_w[1] - cls_w[0]).reshape(4, 128).T),
        "ones_row": np.ones((1, 128), np.float32),
        "ones_blk": np.ones((128, JC), np.float32),
        "id128": np.eye(128, dtype=np.float32),
        "wobb": np.full((128, 1), Wo_b[1] - Wo_b[0], np.float32),
        "clsbb": np.full((128, 1), cls_b[1] - cls_b[0], np.float32),
        "wobbn": np.full((128, 1), Wo_b[0] - Wo_b[1], np.float32),
        "clsbbn": np.full((128, 1), cls_b[0] - cls_b[1], np.float32),
    }
    emoT = text_emo[:, 0, :].T.copy()      # [512, 256]
    in_maps = []
    for c in range(NCORES):
        m = dict(shared)
        m["temoT"] = _chunk(np.ascontiguousarray(
            emoT[:, c * JC:(c + 1) * JC]))
        in_maps.append(m)

    res = run_bass_kernel_spmd(nc, in_maps, core_ids=list(range(NCORES)),
                               trace=TRACE)
    _CACHE["last_res"] = res

    p2 = np.concatenate([res.results[c]["p2out"] for c in range(NCORES)], 0)
    p3d = np.concatenate(
        [res.results[c]["p3out"].reshape(JC, K, CS, 2)
         for c in range(NCORES)], 0)           # [256, 64, 8, 2]

    sel_j, sel_k = np.nonzero(label_ck)
    logit_out = np.ascontiguousarray(
        p3d[sel_j, sel_k].reshape(-1, 2), dtype=np.float32)

    l3 = label3.reshape(SE, -1)
    Lcau = np.zeros((SE, SC), np.float32)
    rows = np.repeat(np.arange(SE), l3.shape[1])
    cols = np.asarray(l3, np.int64).ravel()
    valid = cols >= 0
    Lcau[rows[valid], cols[valid]] = 1.0
    Lck = Lcau.reshape(SE, K, CS)
    label3_out = Lck[sel_j, sel_k].reshape(-1).astype(np.int32)

    return (np.asarray(p2, np.float32), logit_out, label3_out)


# revision 25
# speedup vs baseline: 1.1294x; 1.1294x over previous
"""Trainium2 Bass kernel for nn_ECPEC (emotion-cause pair extraction).

Sharding: data-parallel over the 256 emotion utterances j -> 32 rows per
NeuronCore; cause-side encodings (rep layer, GRU chunk encoder) and all
weights are replicated on every core.  The device computes DENSE outputs
over all (j, k) chunk pairs; the ragged teacher-forcing selection
(nonzeros of label_ck) and label3_out are pure host-side indexing, so the
compiled program is input-shape independent.

Math restructuring vs. the reference graph (all matmuls in float32r):
  phase2:  sigmoid([emo, ch] @ Ww.T + b) == sigmoid(W1@emo.T + W2@ch.T + b)
  phase3:  delta @ W3.T == We@e_j + Wt@c_s + wn*||e_j-c_s|| + Wet@(e_j*c_s) + b3
           with ||e-c||^2 == e2 - 2 e.c + c2 computed via Gram matmuls.
           Wt@c / wn-outer-product / bias terms are accumulated into the
           same PSUM group via identity / rank-1 matmuls; We@e_j + b3 is
           the per-partition bias of the Relu activation.
  log_softmax over 2 classes: d = x1 - x0;
           out = (-softplus(d), d - softplus(d)) via ScalarE Softplus.
"""

from contextlib import ExitStack

import numpy as np

import concourse.bass as bass
import concourse.tile as tile
from concourse import bacc, mybir
from concourse.bass_utils import run_bass_kernel_spmd

D = 512
SE = 256
SC = 512
CS = 8
K = 64              # number of cause chunks
NCORES = 8
JC = SE // NCORES   # 32 emotion rows per core
COLS = JC * SC      # 16384 dense phase3 rows per core ((j,k,t) = (j,s))
P2 = JC * K         # 2048 dense phase2 rows per core

FR = mybir.dt.float32r
F32 = mybir.dt.float32
AF = mybir.ActivationFunctionType
OP = mybir.AluOpType
ts = bass.ts
ds = bass.ds


def _ap(base, ap_list, extra_offset=0):
    return bass.AP(tensor=base.tensor, offset=base.offset + extra_offset,
                   ap=[list(x) for x in ap_list])


def build_nc():
    nc = bacc.Bacc("TRN2", target_bir_lowering=False, debug=False,
                   num_devices=NCORES)

    names_fr = {
        "temoT": [128, 4 * JC], "tcauT": [128, 4 * SC],
        "repT": [128, 4 * D], "w1T": [128, 4 * D], "w2T": [128, 4 * D],
        "wb_row": [1, D], "wod": [128, 4],
        "wihT": [128, 4 * 3 * D], "whhT": [128, 4 * 3 * D],
        "grub_row": [1, 3 * D], "bhn_row": [1, D],
        "weT": [128, 4 * D], "wtT": [128, 4 * D], "wetT": [128, 4 * D],
        "wn_row": [1, D], "b3_row": [1, D], "clsd": [128, 4],
        "ones_row": [1, 128], "ones_blk": [128, JC], "id128": [128, 128],
    }
    dram = {}
    for n, shp in names_fr.items():
        dram[n] = nc.dram_tensor(n, shp, FR, kind="ExternalInput")
    for n in ("rep_b", "wobb", "clsbb", "wobbn", "clsbbn"):
        dram[n] = nc.dram_tensor(n, [128, 4] if n == "rep_b" else [128, 1],
                                 F32, kind="ExternalInput")
    dram["p2out"] = nc.dram_tensor("p2out", [P2, 2], F32, kind="ExternalOutput")
    dram["p3out"] = nc.dram_tensor("p3out", [COLS, 2], F32,
                                   kind="ExternalOutput")

    with tile.TileContext(nc, pool_alloc_mode="queue") as tc:
        _emit(tc, dram)
    nc.compile()
    return nc


def _emit(tc, dram):
    nc = tc.nc
    ctx = ExitStack()
    with ctx:
        res = ctx.enter_context(tc.tile_pool(name="res", bufs=1))
        ps_z = ctx.enter_context(tc.tile_pool(name="ps_z", bufs=3,
                                              space="PSUM"))
        ps_gru = ctx.enter_context(tc.tile_pool(name="ps_gru", bufs=1,
                                                space="PSUM"))
        ps_d = ctx.enter_context(tc.tile_pool(name="ps_d", bufs=1,
                                              space="PSUM"))

        gxp = ctx.enter_context(tc.tile_pool(name="gxp", bufs=1))
        gx3 = gxp.tile([K, CS, 3 * D], FR, tag="gx3")
        w0_cm = tc.tile_pool(name="w0", bufs=1)
        w0 = w0_cm.__enter__()

        def load(pool, name, shape, dt=FR, eng=None):
            tl = pool.tile(shape, dt, tag=name)
            (eng or nc.sync).dma_start(tl[:], dram[name].ap())
            return tl

        # critical-path loads first (idn feeds the PE warm-up spin)
        idn = load(res, "id128", [128, 128])
        rep = load(w0, "repT", [128, 4, D])
        repb = load(w0, "rep_b", [128, 4], F32)
        tem = load(w0, "temoT", [128, 4, JC])
        tca = load(w0, "tcauT", [128, 4, SC])
        wtw = load(w0, "wtT", [128, 4, D])
        wih = load(w0, "wihT", [128, 4, 3 * D])
        we = load(w0, "weT", [128, 4, D])
        w1 = load(w0, "w1T", [128, 4, D])
        wet = load(res, "wetT", [128, 4, D], eng=nc.scalar)

        # resident tensors
        whh = load(res, "whhT", [128, 4, 3 * D], eng=nc.scalar)
        w2 = load(res, "w2T", [128, 4, D], eng=nc.scalar)
        wnr = load(res, "wn_row", [1, D])
        bhnr = load(res, "bhn_row", [1, D])
        onesr = load(res, "ones_row", [1, 128])
        onesb = load(res, "ones_blk", [128, JC])
        clsd_s = load(res, "clsd", [128, 4])
        wod_s = load(res, "wod", [128, 4])
        wobb = load(res, "wobb", [128, 1], F32)
        clsbb = load(res, "clsbb", [128, 1], F32)
        wobbn = load(res, "wobbn", [128, 1], F32)
        clsbbn = load(res, "clsbbn", [128, 1], F32)

        eT = res.tile([128, 4, JC], FR, tag="eT")
        cT = res.tile([128, 4, SC], FR, tag="cT")
        cauW = res.tile([128, 4, SC], FR, tag="cauW")
        ejb = res.tile([128, 4, JC], F32, tag="ejb")
        atb = res.tile([128, 4, JC], F32, tag="atb")
        btk = res.tile([128, 4, K], F32, tag="btk")
        e2T = res.tile([JC, 1], F32, tag="e2T")
        nrm32 = res.tile([JC, SC], FR, tag="nrm32")

        # remaining one-shot weights
        wbr = load(w0, "wb_row", [1, D])
        b3r = load(w0, "b3_row", [1, D])
        grubr = load(w0, "grub_row", [1, 3 * D])



        # ================= setup =========================================
        # PE warm-up spin: keep TensorE busy through the DMA-in phase so the
        # HAM clock gate opens before real work starts (write-only psum).
        warm = ps_z.tile([128, 128], F32, tag="z")
        for _ in range(72):
            nc.tensor.matmul(warm[:], idn[:, :], idn[:, :],
                             start=True, stop=True)

        # rep layer: e = relu(W@emo.T + b), c = relu(W@cau.T + b)
        for m in range(4):
            pp = ps_z.tile([128, JC], F32, tag="z")
            for k in range(4):
                nc.tensor.matmul(pp[:], rep[:, k, ts(m, 128)],
                                 tem[:, k, :], start=(k == 0), stop=(k == 3))
            nc.scalar.activation(eT[:, m, :], pp[:], AF.Relu,
                                 bias=repb[:, m:m + 1], scale=1.0)
        for m in range(4):
            pp = ps_z.tile([128, SC], F32, tag="z")
            for k in range(4):
                nc.tensor.matmul(pp[:], rep[:, k, ts(m, 128)],
                                 tca[:, k, :], start=(k == 0), stop=(k == 3))
            nc.scalar.activation(cT[:, m, :], pp[:], AF.Relu,
                                 bias=repb[:, m:m + 1], scale=1.0)

        # cauW = Wt @ cau
        for m in range(4):
            pp = ps_z.tile([128, SC], F32, tag="z")
            for k in range(4):
                nc.tensor.matmul(pp[:], wtw[:, k, ts(m, 128)],
                                 cT[:, k, :], start=(k == 0), stop=(k == 3))
            nc.vector.tensor_copy(cauW[:, m, :], pp[:])

        # ||e_j - c_s||: via e2 - 2 e.c + c2 (DVE runs under the PE setup)
        sq_c = w0.tile([128, 4, SC], FR, tag="sqc")
        nc.vector.tensor_mul(sq_c[:], cT[:], cT[:])
        sq_e = w0.tile([128, 4, JC], FR, tag="sqe")
        nc.vector.tensor_mul(sq_e[:], eT[:], eT[:])
        em2 = w0.tile([128, 4, JC], FR, tag="em2")
        nc.vector.tensor_scalar_mul(em2[:], eT[:], -2.0)

        e2ps = ps_d.tile([1, JC], F32, tag="d")
        for k in range(4):
            nc.tensor.matmul(e2ps[:], onesb[:, 0:1], sq_e[:, k, :],
                             start=(k == 0), stop=(k == 3))
        e2sb = w0.tile([1, JC], F32, tag="e2sb")
        nc.vector.tensor_copy(e2sb[:], e2ps[:])
        nc.sync.dma_start(e2T[:], e2sb[:])          # [1,32] -> [32,1]

        dot = ps_z.tile([JC, SC], F32, tag="z")
        for k in range(4):
            nc.tensor.matmul(dot[:], em2[:, k, :], cT[:, k, :],
                             start=(k == 0), stop=False)
        for k in range(4):
            nc.tensor.matmul(dot[:], onesb[:, :], sq_c[:, k, :],
                             start=False, stop=(k == 3))
        nsq = w0.tile([JC, SC], F32, tag="nsq")
        nc.vector.tensor_scalar(nsq[:], dot[:], e2T[:], 0.0, OP.add, OP.max)
        nc.scalar.activation(nrm32[:], nsq[:], AF.Sqrt)

        def gx_block(m):
            # GX rows 128m..128m+128 of cau @ wih.T + grub -> gx3[k, t, 3D].
            # Rotate psum between pools (5 effective slots) and drain on the
            # idle ScalarE so the PE never waits for a slot.
            for g in range(3):
                i = m * 3 + g
                if i % 5 == 3:
                    pp = ps_gru.tile([128, D], F32, tag="gh")
                elif i % 5 == 4:
                    pp = ps_gru.tile([128, D], F32, tag="tp")
                else:
                    pp = ps_z.tile([128, D], F32, tag="z")
                for k in range(4):
                    nc.tensor.matmul(pp[:], cT[:, k, ts(m, 128)],
                                     wih[:, k, ds(g * D, D)],
                                     start=(k == 0), stop=False)
                nc.tensor.matmul(pp[:], onesr[0:1, 0:128],
                                 grubr[0:1, ds(g * D, D)],
                                 start=False, stop=True)
                stg = w0.tile([128, D], FR, tag="gxstg")
                nc.scalar.activation(stg[:], pp[:], AF.Copy)
                nc.sync.dma_start(
                    gx3[16 * m:16 * (m + 1), :, ds(g * D, D)], stg[:])

        def ejb_block(m):
            pp = ps_z.tile([128, JC], F32, tag="z")
            for k in range(4):
                nc.tensor.matmul(pp[:], we[:, k, ts(m, 128)],
                                 eT[:, k, :], start=(k == 0), stop=False)
            nc.tensor.matmul(pp[:], b3r[0:1, ts(m, 128)],
                             onesr[0:1, 0:JC], start=False, stop=True)
            nc.vector.tensor_copy(ejb[:, m, :], pp[:])

        def atb_block(m):
            pp = ps_z.tile([128, JC], F32, tag="z")
            for k in range(4):
                nc.tensor.matmul(pp[:], w1[:, k, ts(m, 128)],
                                 eT[:, k, :], start=(k == 0), stop=False)
            nc.tensor.matmul(pp[:], wbr[0:1, ts(m, 128)],
                             onesr[0:1, 0:JC], start=False, stop=True)
            nc.vector.tensor_copy(atb[:, m, :], pp[:])

        for m in range(4):
            gx_block(m)
        for m in range(4):
            ejb_block(m)
        for m in range(4):
            atb_block(m)


        w0_cm.__exit__(None, None, None)

        # ============ main loop: phase3 over j, GRU + phase2 woven in ====
        scr1a = ctx.enter_context(tc.tile_pool(name="scr1a", bufs=1))
        scr1b = ctx.enter_context(tc.tile_pool(name="scr1b", bufs=2))
        scr2 = ctx.enter_context(tc.tile_pool(name="scr2", bufs=2))
        scr3 = ctx.enter_context(tc.tile_pool(name="scr3", bufs=1))
        nrmp = ctx.enter_context(tc.tile_pool(name="nrmp", bufs=4))
        fin = ctx.enter_context(tc.tile_pool(name="fin", bufs=1))
        sd2 = fin.tile([4, D], F32, tag="sd2")
        sd3 = fin.tile([JC, SC], F32, tag="sd3")

        hrow = scr1b.tile([K, D], F32, tag="hrow")
        nc.vector.memset(hrow[:], 0.0)
        hT = None

        def gru_step(st):
            nonlocal hrow, hT
            gh = ps_gru.tile([K, 3 * D], F32, tag="gh")
            for g in range(3):
                sl = ds(g * D, D)
                if g < 2:
                    nc.tensor.matmul(gh[:, sl], idn[0:K, 0:K],
                                     gx3[:, st, sl], start=True,
                                     stop=(st == 0))
                else:
                    nc.tensor.matmul(gh[:, sl], onesr[0:1, 0:K],
                                     bhnr[0:1, :], start=True, stop=(st == 0))
                if st > 0:
                    for k in range(4):
                        nc.tensor.matmul(gh[:, sl], hT[:, ts(k, K)],
                                         whh[:, k, sl], start=False,
                                         stop=(k == 3))
            rz = scr1a.tile([K, 2 * D], F32, tag="rz")
            nc.scalar.activation(rz[:], gh[:, 0:2 * D], AF.Sigmoid)
            t2 = scr1a.tile([K, D], F32, tag="t2")
            nc.vector.tensor_mul(t2[:], gh[:, 2 * D:3 * D], rz[:, 0:D])
            # SBUF-only steps go to the (otherwise idle) GPSIMD engine so the
            # serial GRU chain does not block the DVE queue feeding phase 3.
            t3 = scr1a.tile([K, D], F32, tag="t3")
            nc.vector.tensor_add(t3[:], t2[:], gx3[:, st, 2 * D:3 * D])
            n_t = scr1a.tile([K, D], F32, tag="n_t")
            nc.scalar.activation(n_t[:], t3[:], AF.Tanh)
            t4 = scr1a.tile([K, D], F32, tag="t4")
            nc.vector.tensor_sub(t4[:], hrow[:], n_t[:])
            t5 = scr1a.tile([K, D], F32, tag="t5")
            nc.vector.tensor_mul(t5[:], rz[:, D:2 * D], t4[:])
            hrow = scr1b.tile([K, D], FR, tag="hrow")
            nc.vector.tensor_add(hrow[:], n_t[:], t5[:])
            tp = ps_gru.tile([128, 4 * K], FR, tag="tp")
            for mc in range(4):
                nc.tensor.transpose(tp[:, ts(mc, K)], hrow[:, ts(mc, 128)],
                                    idn[0:K, 0:K])
            hT = scr1b.tile([128, 4 * K], FR, tag="hT")
            nc.vector.tensor_copy(hT[:], tp[:])

        def btk_calc():
            for m in range(4):
                pp = ps_z.tile([128, K], F32, tag="z")
                for k in range(4):
                    nc.tensor.matmul(pp[:], w2[:, k, ts(m, 128)],
                                     hT[:, ts(k, K)], start=(k == 0),
                                     stop=(k == 3))
                nc.vector.tensor_copy(btk[:, m, :], pp[:])

        p2hid = {}

        def phase2_produce(g):
            hidc = scr3.tile([128, 4, D], FR, tag="hidc")
            for m in range(4):
                tt = scr2.tile([128, D], F32, tag="p2t")
                a0 = atb[:, m, ds(8 * g, 8)]
                b0 = btk[:, m, :]
                a_ap = _ap(a0, [a0.ap[0], a0.ap[1], [0, K]])
                b_ap = _ap(b0, [b0.ap[0], [0, 8], b0.ap[1]])
                nc.vector.tensor_tensor(tt[:], a_ap, b_ap, OP.add)
                nc.scalar.activation(hidc[:, m, :], tt[:], AF.Sigmoid)
            p2hid[g] = hidc

        def phase2_mm(g):
            hidc = p2hid.pop(g)
            dp = ps_d.tile([1, D], F32, tag="d")
            for m in range(4):
                nc.tensor.matmul(dp[:], wod_s[:, m:m + 1], hidc[:, m, :],
                                 start=(m == 0), stop=(m == 3))
            dstg = scr2.tile([1, D], F32, tag="dstg")
            nc.vector.tensor_copy(dstg[:], dp[:])
            nc.sync.dma_start(sd2[g:g + 1, :], dstg[:])

        def fetch_nrm(j):
            nrmj = nrmp.tile([1, SC], FR, tag="nrmj")
            nc.gpsimd.dma_start(nrmj[:], nrm32[j:j + 1, :])
            return nrmj

        def build_xt(j):
            xt = scr2.tile([128, 4, SC], FR, tag="xt")
            for m in range(4):
                nc.vector.tensor_scalar_mul(
                    xt[:, m, :], cT[:, m, :],
                    eT[:, m, j:j + 1].bitcast(F32))
            return xt

        nrm_q = [fetch_nrm(0), fetch_nrm(1), fetch_nrm(2)]
        nxt = build_xt(0)
        for j in range(JC):
            xt = nxt
            nrmj = nrm_q.pop(0)
            if j + 3 < JC:
                nrm_q.append(fetch_nrm(j + 3))
            hj = scr3.tile([128, 4, SC], FR, tag="hj")
            for m in range(4):
                zp = ps_z.tile([128, SC], F32, tag="z")
                for k in range(4):
                    nc.tensor.matmul(zp[:], wet[:, k, ts(m, 128)],
                                     xt[:, k, :], start=(k == 0), stop=False)
                nc.tensor.matmul(zp[:], wnr[0:1, ts(m, 128)],
                                 nrmj[0:1, :], start=False, stop=True)
                hp = scr2.tile([128, SC], F32, tag="hpre")
                nc.vector.scalar_tensor_tensor(
                    hp[:], zp[:], ejb[:, m, j:j + 1], cauW[:, m, :],
                    OP.add, OP.add)
                nc.scalar.activation(hj[:, m, :], hp[:], AF.Relu)

            # next iteration's rhs BEFORE this iteration's d-projection: the
            # dstg psum-drain copy below waits on the d matmuls, and anything
            # queued after it on the in-order DVE stream would stall with it
            # — building xt[j+1] first keeps the PE fed.
            if j + 1 < JC:
                nxt = build_xt(j + 1)

            dp = ps_d.tile([1, SC], F32, tag="d")
            for k in range(4):
                nc.tensor.matmul(dp[:], clsd_s[:, k:k + 1], hj[:, k, :],
                                 start=(k == 0), stop=(k == 3))
            dstg = scr2.tile([1, SC], F32, tag="dstg")
            nc.vector.tensor_copy(dstg[:], dp[:])
            nc.sync.dma_start(sd3[j:j + 1, :], dstg[:])


            # weave the serial GRU + small phase2 blocks into the PE stream
            if j >= 4 and j % 2 == 0 and (j - 4) // 2 < 8:
                gru_step((j - 4) // 2)
                if (j - 4) // 2 == 7:
                    btk_calc()
            if j in (20, 22, 24, 26):
                phase2_produce((j - 20) // 2)
            if j in (22, 24, 26, 28):
                phase2_mm((j - 22) // 2)

        # phase2 log-softmax tail
        sg2 = fin.tile([4, D], F32, tag="sg2")
        nc.scalar.activation(sg2[:], sd2[:], AF.Sigmoid,
                             bias=wobbn[0:4, :], scale=-1.0)
        o2 = fin.tile([4, 2 * D], F32, tag="o2")
        o2e = _ap(o2[:], [o2[:].ap[0], [2, D]])
        nc.scalar.activation(o2e, sg2[:], AF.Ln)
        nc.vector.scalar_tensor_tensor(
            _ap(o2[:], [o2[:].ap[0], [2, D]], extra_offset=1),
            sd2[:], wobb[0:4, :], o2e, OP.add, OP.add)
        nc.sync.dma_start(dram["p2out"].ap(), o2[:])

        # phase3 log-softmax tail
        sg3 = fin.tile([JC, SC], F32, tag="sg3")
        nc.scalar.activation(sg3[:], sd3[:], AF.Sigmoid,
                             bias=clsbbn[0:JC, :], scale=-1.0)
        o3 = fin.tile([JC, 2 * SC], F32, tag="o3")
        o3e = _ap(o3[:], [o3[:].ap[0], [2, SC]])
        nc.scalar.activation(o3e, sg3[:], AF.Ln)
        nc.vector.scalar_tensor_tensor(
            _ap(o3[:], [o3[:].ap[0], [2, SC]], extra_offset=1),
            sd3[:], clsbb[0:JC, :], o3e, OP.add, OP.add)
        nc.sync.dma_start(dram["p3out"].ap(), o3[:])


# ------------------------------------------------------------------ host ----

def _chunk(w):
    """[512, N] -> [128, 4, N] flattened as [128, 4*N] (partition-chunked)."""
    n = w.shape[1]
    return np.ascontiguousarray(
        w.reshape(4, 128, n).transpose(1, 0, 2).reshape(128, 4 * n))


_CACHE = {}
TRACE = False          # test harness sets True to collect NTFF exec time


def kernel(text_emo, text_cau, label_ck, label3, chunksize, t_ratio,
           rep_w, rep_b, W_w, W_b, Wo_w, Wo_b,
           gru_wih, gru_whh, gru_bih, gru_bhh, W3_w, W3_b, cls_w, cls_b):
    text_emo = np.asarray(text_emo, np.float32)
    text_cau = np.asarray(text_cau, np.float32)
    label_ck = np.asarray(label_ck)
    label3 = np.asarray(label3)
    assert int(chunksize) == CS and int(t_ratio) == 1
    f32 = lambda x: np.asarray(x, np.float32)
    rep_w, rep_b, W_w, W_b, Wo_w, Wo_b = map(
        f32, (rep_w, rep_b, W_w, W_b, Wo_w, Wo_b))
    gru_wih, gru_whh, gru_bih, gru_bhh = map(
        f32, (gru_wih, gru_whh, gru_bih, gru_bhh))
    W3_w, W3_b, cls_w, cls_b = map(f32, (W3_w, W3_b, cls_w, cls_b))

    if "nc" not in _CACHE:
        _CACHE["nc"] = build_nc()
    nc = _CACHE["nc"]

    grub = np.concatenate([gru_bih[:2 * D] + gru_bhh[:2 * D],
                           gru_bih[2 * D:]])
    shared = {
        "tcauT": _chunk(text_cau[:, 0, :].T.copy()),
        "repT": _chunk(rep_w.T.copy()),
        "rep_b": np.ascontiguousarray(rep_b.reshape(4, 128).T),
        "w1T": _chunk(W_w[:, :D].T.copy()),
        "w2T": _chunk(W_w[:, D:].T.copy()),
        "wb_row": W_b.reshape(1, D).copy(),
        "wod": np.ascontiguousarray((Wo_w[1] - Wo_w[0]).reshape(4, 128).T),
        "wihT": _chunk(gru_wih.T.copy()),
        "whhT": _chunk(gru_whh.T.copy()),
        "grub_row": grub.reshape(1, 3 * D),
        "bhn_row": gru_bhh[2 * D:].reshape(1, D).copy(),
        "weT": _chunk(W3_w[:, :D].T.copy()),
        "wtT": _chunk(W3_w[:, D:2 * D].T.copy()),
        "wetT": _chunk(W3_w[:, 2 * D + 1:].T.copy()),
        "wn_row": W3_w[:, 2 * D].reshape(1, D).copy(),
        "b3_row": W3_b.reshape(1, D).copy(),
        "clsd": np.ascontiguousarray((cls_w[1] - cls_w[0]).reshape(4, 128).T),
        "ones_row": np.ones((1, 128), np.float32),
        "ones_blk": np.ones((128, JC), np.float32),
        "id128": np.eye(128, dtype=np.float32),
        "wobb": np.full((128, 1), Wo_b[1] - Wo_b[0], np.float32),
        "clsbb": np.full((128, 1), cls_b[1] - cls_b[0], np.float32),
        "wobbn": np.full((128, 1), Wo_b[0] - Wo_b[1], np.float32),
        "clsbbn": np.full((128, 1), cls_b[0] - cls_b[1], np.float32),
    }
    emoT = text_emo[:, 0, :].T.copy()      # [512, 256]
    in_maps = []
    for c in range(NCORES):
        m = dict(shared)
        m["temoT"] = _chunk(np.ascontiguousarray(
            emoT[:, c * JC:(c + 1) * JC]))
        in_maps.append(m)

    res = run_bass_kernel_spmd(nc, in_maps, core_ids=list(range(NCORES)),
                               trace=TRACE)
    _CACHE["last_res"] = res

    p2 = np.concatenate([res.results[c]["p2out"] for c in range(NCORES)], 0)
    p3d = np.concatenate(
        [res.results[c]["p3out"].reshape(JC, K, CS, 2)
         for c in range(NCORES)], 0)           # [256, 64, 8, 2]

    sel_j, sel_k = np.nonzero(label_ck)
    logit_out = np.ascontiguousarray(
        p3d[sel_j, sel_k].reshape(-1, 2), dtype=np.float32)

    l3 = label3.reshape(SE, -1)
    Lcau = np.zeros((SE, SC), np.float32)
    rows = np.repeat(np.arange(SE), l3.shape[1])
    cols = np.asarray(l3, np.int64).ravel()
    valid = cols >= 0
    Lcau[rows[valid], cols[valid]] = 1.0
    Lck = Lcau.reshape(SE, K, CS)
    label3_out = Lck[sel_j, sel_k].reshape(-1).astype(np.int32)

    return (np.asarray(p2, np.float32), logit_out, label3_out)


# revision 27
# speedup vs baseline: 1.1342x; 1.0042x over previous
"""Trainium2 Bass kernel for nn_ECPEC (emotion-cause pair extraction).

Sharding: data-parallel over the 256 emotion utterances j -> 32 rows per
NeuronCore; cause-side encodings (rep layer, GRU chunk encoder) and all
weights are replicated on every core.  The device computes DENSE outputs
over all (j, k) chunk pairs; the ragged teacher-forcing selection
(nonzeros of label_ck) and label3_out are pure host-side indexing, so the
compiled program is input-shape independent.

Math restructuring vs. the reference graph (all matmuls in float32r):
  phase2:  sigmoid([emo, ch] @ Ww.T + b) == sigmoid(W1@emo.T + W2@ch.T + b)
  phase3:  delta @ W3.T == We@e_j + Wt@c_s + wn*||e_j-c_s|| + Wet@(e_j*c_s) + b3
           with ||e-c||^2 == e2 - 2 e.c + c2 computed via Gram matmuls.
           Wt@c / wn-outer-product / bias terms are accumulated into the
           same PSUM group via identity / rank-1 matmuls; We@e_j + b3 is
           the per-partition bias of the Relu activation.
  log_softmax over 2 classes: d = x1 - x0;
           out = (-softplus(d), d - softplus(d)) via ScalarE Softplus.
"""

from contextlib import ExitStack

import numpy as np

import concourse.bass as bass
import concourse.tile as tile
from concourse import bacc, mybir
from concourse.bass_utils import run_bass_kernel_spmd

D = 512
SE = 256
SC = 512
CS = 8
K = 64              # number of cause chunks
NCORES = 8
JC = SE // NCORES   # 32 emotion rows per core
COLS = JC * SC      # 16384 dense phase3 rows per core ((j,k,t) = (j,s))
P2 = JC * K         # 2048 dense phase2 rows per core

FR = mybir.dt.float32r
F32 = mybir.dt.float32
AF = mybir.ActivationFunctionType
OP = mybir.AluOpType
ts = bass.ts
ds = bass.ds


def _ap(base, ap_list, extra_offset=0):
    return bass.AP(tensor=base.tensor, offset=base.offset + extra_offset,
                   ap=[list(x) for x in ap_list])


def build_nc():
    nc = bacc.Bacc("TRN2", target_bir_lowering=False, debug=False,
                   num_devices=NCORES)

    names_fr = {
        "temoT": [128, 4 * JC], "tcauT": [128, 4 * SC],
        "repT": [128, 4 * D], "w1T": [128, 4 * D], "w2T": [128, 4 * D],
        "wb_row": [1, D], "wod": [128, 4],
        "wihT": [128, 4 * 3 * D], "whhT": [128, 4 * 3 * D],
        "grub_row": [1, 3 * D], "bhn_row": [1, D],
        "weT": [128, 4 * D], "wtT": [128, 4 * D], "wetT": [128, 4 * D],
        "wn_row": [1, D], "b3_row": [1, D], "clsd": [128, 4],
        "ones_row": [1, 128], "ones_blk": [128, JC], "id128": [128, 128],
    }
    dram = {}
    for n, shp in names_fr.items():
        dram[n] = nc.dram_tensor(n, shp, FR, kind="ExternalInput")
    for n in ("rep_b", "wobb", "clsbb", "wobbn", "clsbbn"):
        dram[n] = nc.dram_tensor(n, [128, 4] if n == "rep_b" else [128, 1],
                                 F32, kind="ExternalInput")
    dram["p2out"] = nc.dram_tensor("p2out", [P2, 2], F32, kind="ExternalOutput")
    dram["p3out"] = nc.dram_tensor("p3out", [COLS, 2], F32,
                                   kind="ExternalOutput")

    with tile.TileContext(nc, pool_alloc_mode="queue") as tc:
        _emit(tc, dram)
    nc.compile()
    return nc


def _emit(tc, dram):
    nc = tc.nc
    ctx = ExitStack()
    with ctx:
        res = ctx.enter_context(tc.tile_pool(name="res", bufs=1))
        ps_z = ctx.enter_context(tc.tile_pool(name="ps_z", bufs=3,
                                              space="PSUM"))
        ps_gru = ctx.enter_context(tc.tile_pool(name="ps_gru", bufs=1,
                                                space="PSUM"))
        ps_d = ctx.enter_context(tc.tile_pool(name="ps_d", bufs=1,
                                              space="PSUM"))

        gxp = ctx.enter_context(tc.tile_pool(name="gxp", bufs=1))
        gx3 = gxp.tile([K, CS, 3 * D], FR, tag="gx3")
        w0_cm = tc.tile_pool(name="w0", bufs=1)
        w0 = w0_cm.__enter__()

        def load(pool, name, shape, dt=FR, eng=None):
            tl = pool.tile(shape, dt, tag=name)
            (eng or nc.sync).dma_start(tl[:], dram[name].ap())
            return tl

        # critical-path loads first (idn feeds the PE warm-up spin)
        idn = load(res, "id128", [128, 128])
        rep = load(w0, "repT", [128, 4, D])
        repb = load(w0, "rep_b", [128, 4], F32)
        tem = load(w0, "temoT", [128, 4, JC])
        tca = load(w0, "tcauT", [128, 4, SC])
        wtw = load(w0, "wtT", [128, 4, D])
        wih = load(w0, "wihT", [128, 4, 3 * D])
        we = load(w0, "weT", [128, 4, D])
        w1 = load(w0, "w1T", [128, 4, D])
        wet = load(res, "wetT", [128, 4, D], eng=nc.scalar)

        # resident tensors
        whh = load(res, "whhT", [128, 4, 3 * D], eng=nc.scalar)
        w2 = load(res, "w2T", [128, 4, D], eng=nc.scalar)
        wnr = load(res, "wn_row", [1, D])
        bhnr = load(res, "bhn_row", [1, D])
        onesr = load(res, "ones_row", [1, 128])
        onesb = load(res, "ones_blk", [128, JC])
        clsd_s = load(res, "clsd", [128, 4])
        wod_s = load(res, "wod", [128, 4])
        wobb = load(res, "wobb", [128, 1], F32)
        clsbb = load(res, "clsbb", [128, 1], F32)
        wobbn = load(res, "wobbn", [128, 1], F32)
        clsbbn = load(res, "clsbbn", [128, 1], F32)

        eT = res.tile([128, 4, JC], FR, tag="eT")
        cT = res.tile([128, 4, SC], FR, tag="cT")
        cauW = res.tile([128, 4, SC], FR, tag="cauW")
        ejb = res.tile([128, 4, JC], F32, tag="ejb")
        atb = res.tile([128, 4, JC], F32, tag="atb")
        btk = res.tile([128, 4, K], F32, tag="btk")
        e2T = res.tile([JC, 1], F32, tag="e2T")
        nrm32 = res.tile([JC, SC], FR, tag="nrm32")

        # remaining one-shot weights
        wbr = load(w0, "wb_row", [1, D])
        b3r = load(w0, "b3_row", [1, D])
        grubr = load(w0, "grub_row", [1, 3 * D])



        # ================= setup =========================================
        # PE warm-up spin: keep TensorE busy through the DMA-in phase so the
        # HAM clock gate opens before real work starts (write-only psum).
        warm = ps_z.tile([128, 128], F32, tag="z")
        for _ in range(72):
            nc.tensor.matmul(warm[:], idn[:, :], idn[:, :],
                             start=True, stop=True)

        # rep layer: e = relu(W@emo.T + b), c = relu(W@cau.T + b)
        for m in range(4):
            pp = ps_z.tile([128, JC], F32, tag="z")
            for k in range(4):
                nc.tensor.matmul(pp[:], rep[:, k, ts(m, 128)],
                                 tem[:, k, :], start=(k == 0), stop=(k == 3))
            nc.scalar.activation(eT[:, m, :], pp[:], AF.Relu,
                                 bias=repb[:, m:m + 1], scale=1.0)
        for m in range(4):
            pp = ps_z.tile([128, SC], F32, tag="z")
            for k in range(4):
                nc.tensor.matmul(pp[:], rep[:, k, ts(m, 128)],
                                 tca[:, k, :], start=(k == 0), stop=(k == 3))
            nc.scalar.activation(cT[:, m, :], pp[:], AF.Relu,
                                 bias=repb[:, m:m + 1], scale=1.0)

        # cauW = Wt @ cau
        for m in range(4):
            pp = ps_z.tile([128, SC], F32, tag="z")
            for k in range(4):
                nc.tensor.matmul(pp[:], wtw[:, k, ts(m, 128)],
                                 cT[:, k, :], start=(k == 0), stop=(k == 3))
            nc.vector.tensor_copy(cauW[:, m, :], pp[:])

        # ||e_j - c_s||: via e2 - 2 e.c + c2 (DVE runs under the PE setup)
        sq_c = w0.tile([128, 4, SC], FR, tag="sqc")
        nc.vector.tensor_mul(sq_c[:], cT[:], cT[:])
        sq_e = w0.tile([128, 4, JC], FR, tag="sqe")
        nc.vector.tensor_mul(sq_e[:], eT[:], eT[:])
        em2 = w0.tile([128, 4, JC], FR, tag="em2")
        nc.vector.tensor_scalar_mul(em2[:], eT[:], -2.0)

        e2ps = ps_d.tile([1, JC], F32, tag="d")
        for k in range(4):
            nc.tensor.matmul(e2ps[:], onesb[:, 0:1], sq_e[:, k, :],
                             start=(k == 0), stop=(k == 3))
        e2sb = w0.tile([1, JC], F32, tag="e2sb")
        nc.vector.tensor_copy(e2sb[:], e2ps[:])
        nc.sync.dma_start(e2T[:], e2sb[:])          # [1,32] -> [32,1]

        dot = ps_z.tile([JC, SC], F32, tag="z")
        for k in range(4):
            nc.tensor.matmul(dot[:], em2[:, k, :], cT[:, k, :],
                             start=(k == 0), stop=False)
        for k in range(4):
            nc.tensor.matmul(dot[:], onesb[:, :], sq_c[:, k, :],
                             start=False, stop=(k == 3))
        nsq = w0.tile([JC, SC], F32, tag="nsq")
        nc.vector.tensor_scalar(nsq[:], dot[:], e2T[:], 0.0, OP.add, OP.max)
        nc.scalar.activation(nrm32[:], nsq[:], AF.Sqrt)

        def gx_block(m):
            # GX rows 128m..128m+128 of cau @ wih.T + grub -> gx3[k, t, 3D].
            # Rotate psum between pools (5 effective slots) and drain on the
            # idle ScalarE so the PE never waits for a slot.
            for g in range(3):
                i = m * 3 + g
                if i % 5 == 3:
                    pp = ps_gru.tile([128, D], F32, tag="gh")
                elif i % 5 == 4:
                    pp = ps_gru.tile([128, D], F32, tag="tp")
                else:
                    pp = ps_z.tile([128, D], F32, tag="z")
                for k in range(4):
                    nc.tensor.matmul(pp[:], cT[:, k, ts(m, 128)],
                                     wih[:, k, ds(g * D, D)],
                                     start=(k == 0), stop=False)
                nc.tensor.matmul(pp[:], onesr[0:1, 0:128],
                                 grubr[0:1, ds(g * D, D)],
                                 start=False, stop=True)
                stg = w0.tile([128, D], FR, tag="gxstg")
                nc.scalar.activation(stg[:], pp[:], AF.Copy)
                nc.sync.dma_start(
                    gx3[16 * m:16 * (m + 1), :, ds(g * D, D)], stg[:])

        def ejb_block(m):
            pp = ps_z.tile([128, JC], F32, tag="z")
            for k in range(4):
                nc.tensor.matmul(pp[:], we[:, k, ts(m, 128)],
                                 eT[:, k, :], start=(k == 0), stop=False)
            nc.tensor.matmul(pp[:], b3r[0:1, ts(m, 128)],
                             onesr[0:1, 0:JC], start=False, stop=True)
            nc.vector.tensor_copy(ejb[:, m, :], pp[:])

        def atb_block(m):
            pp = ps_z.tile([128, JC], F32, tag="z")
            for k in range(4):
                nc.tensor.matmul(pp[:], w1[:, k, ts(m, 128)],
                                 eT[:, k, :], start=(k == 0), stop=False)
            nc.tensor.matmul(pp[:], wbr[0:1, ts(m, 128)],
                             onesr[0:1, 0:JC], start=False, stop=True)
            nc.vector.tensor_copy(atb[:, m, :], pp[:])

        for m in range(4):
            gx_block(m)
        for m in range(4):
            ejb_block(m)
        for m in range(4):
            atb_block(m)


        w0_cm.__exit__(None, None, None)

        # ============ main loop: phase3 over j, GRU + phase2 woven in ====
        scr1a = ctx.enter_context(tc.tile_pool(name="scr1a", bufs=1))
        scr1b = ctx.enter_context(tc.tile_pool(name="scr1b", bufs=2))
        scr2 = ctx.enter_context(tc.tile_pool(name="scr2", bufs=2))
        scr3 = ctx.enter_context(tc.tile_pool(name="scr3", bufs=1))
        nrmp = ctx.enter_context(tc.tile_pool(name="nrmp", bufs=4))
        fin = ctx.enter_context(tc.tile_pool(name="fin", bufs=1))
        sd2 = fin.tile([4, D], F32, tag="sd2")
        sd3 = fin.tile([JC, SC], F32, tag="sd3")

        hrow = scr1b.tile([K, D], F32, tag="hrow")
        nc.vector.memset(hrow[:], 0.0)
        hT = None

        def gru_step(st):
            nonlocal hrow, hT
            gh = ps_gru.tile([K, 3 * D], F32, tag="gh")
            for g in range(3):
                sl = ds(g * D, D)
                if g < 2:
                    nc.tensor.matmul(gh[:, sl], idn[0:K, 0:K],
                                     gx3[:, st, sl], start=True,
                                     stop=(st == 0))
                else:
                    nc.tensor.matmul(gh[:, sl], onesr[0:1, 0:K],
                                     bhnr[0:1, :], start=True, stop=(st == 0))
                if st > 0:
                    for k in range(4):
                        nc.tensor.matmul(gh[:, sl], hT[:, ts(k, K)],
                                         whh[:, k, sl], start=False,
                                         stop=(k == 3))
            rz = scr1a.tile([K, 2 * D], F32, tag="rz")
            nc.scalar.activation(rz[:], gh[:, 0:2 * D], AF.Sigmoid)
            t2 = scr1a.tile([K, D], F32, tag="t2")
            nc.vector.tensor_mul(t2[:], gh[:, 2 * D:3 * D], rz[:, 0:D])
            # SBUF-only steps go to the (otherwise idle) GPSIMD engine so the
            # serial GRU chain does not block the DVE queue feeding phase 3.
            t3 = scr1a.tile([K, D], F32, tag="t3")
            nc.vector.tensor_add(t3[:], t2[:], gx3[:, st, 2 * D:3 * D])
            n_t = scr1a.tile([K, D], F32, tag="n_t")
            nc.scalar.activation(n_t[:], t3[:], AF.Tanh)
            t4 = scr1a.tile([K, D], F32, tag="t4")
            nc.vector.tensor_sub(t4[:], hrow[:], n_t[:])
            t5 = scr1a.tile([K, D], F32, tag="t5")
            nc.vector.tensor_mul(t5[:], rz[:, D:2 * D], t4[:])
            hrow = scr1b.tile([K, D], FR, tag="hrow")
            nc.vector.tensor_add(hrow[:], n_t[:], t5[:])
            tp = ps_gru.tile([128, 4 * K], FR, tag="tp")
            for mc in range(4):
                nc.tensor.transpose(tp[:, ts(mc, K)], hrow[:, ts(mc, 128)],
                                    idn[0:K, 0:K])
            hT = scr1b.tile([128, 4 * K], FR, tag="hT")
            nc.scalar.activation(hT[:], tp[:], AF.Copy)

        def btk_calc():
            for m in range(4):
                pp = ps_z.tile([128, K], F32, tag="z")
                for k in range(4):
                    nc.tensor.matmul(pp[:], w2[:, k, ts(m, 128)],
                                     hT[:, ts(k, K)], start=(k == 0),
                                     stop=(k == 3))
                nc.vector.tensor_copy(btk[:, m, :], pp[:])

        p2hid = {}

        def phase2_produce(g):
            hidc = scr3.tile([128, 4, D], FR, tag="hidc")
            for m in range(4):
                tt = scr2.tile([128, D], F32, tag="p2t")
                a0 = atb[:, m, ds(8 * g, 8)]
                b0 = btk[:, m, :]
                a_ap = _ap(a0, [a0.ap[0], a0.ap[1], [0, K]])
                b_ap = _ap(b0, [b0.ap[0], [0, 8], b0.ap[1]])
                nc.vector.tensor_tensor(tt[:], a_ap, b_ap, OP.add)
                nc.scalar.activation(hidc[:, m, :], tt[:], AF.Sigmoid)
            p2hid[g] = hidc

        def phase2_mm(g):
            hidc = p2hid.pop(g)
            dp = ps_d.tile([1, D], F32, tag="d")
            for m in range(4):
                nc.tensor.matmul(dp[:], wod_s[:, m:m + 1], hidc[:, m, :],
                                 start=(m == 0), stop=(m == 3))
            dstg = scr2.tile([1, D], F32, tag="dstg")
            nc.vector.tensor_copy(dstg[:], dp[:])
            nc.sync.dma_start(sd2[g:g + 1, :], dstg[:])

        def fetch_nrm(j):
            nrmj = nrmp.tile([1, SC], FR, tag="nrmj")
            nc.gpsimd.dma_start(nrmj[:], nrm32[j:j + 1, :])
            return nrmj

        def build_xt(j):
            xt = scr2.tile([128, 4, SC], FR, tag="xt")
            for m in range(4):
                nc.vector.tensor_scalar_mul(
                    xt[:, m, :], cT[:, m, :],
                    eT[:, m, j:j + 1].bitcast(F32))
            return xt

        def d_proj(j, hjt):
            # cls difference-logit projection for row-block j (d = h @ clsd)
            dp = ps_d.tile([1, SC], F32, tag="d")
            for k in range(4):
                nc.tensor.matmul(dp[:], clsd_s[:, k:k + 1], hjt[:, k, :],
                                 start=(k == 0), stop=(k == 3))
            dstg = scr2.tile([1, SC], F32, tag="dstg")
            nc.scalar.activation(dstg[:], dp[:], AF.Copy)
            nc.sync.dma_start(sd3[j:j + 1, :], dstg[:])

        nrm_q = [fetch_nrm(0), fetch_nrm(1), fetch_nrm(2)]
        nxt = build_xt(0)
        prev_hj = None
        for j in range(JC):
            xt = nxt
            nrmj = nrm_q.pop(0)
            if j + 3 < JC:
                nrm_q.append(fetch_nrm(j + 3))
            hj = scr3.tile([128, 4, SC], FR, tag="hj")
            for m in range(4):
                zp = ps_z.tile([128, SC], F32, tag="z")
                for k in range(4):
                    nc.tensor.matmul(zp[:], wet[:, k, ts(m, 128)],
                                     xt[:, k, :], start=(k == 0), stop=False)
                nc.tensor.matmul(zp[:], wnr[0:1, ts(m, 128)],
                                 nrmj[0:1, :], start=False, stop=True)
                hp = scr2.tile([128, SC], F32, tag="hpre")
                nc.vector.scalar_tensor_tensor(
                    hp[:], zp[:], ejb[:, m, j:j + 1], cauW[:, m, :],
                    OP.add, OP.add)
                nc.scalar.activation(hj[:, m, :], hp[:], AF.Relu)
                if m == 0:
                    # previous row-block's d-projection sits here, mid-way
                    # through this block's z matmuls, so the PE never waits
                    # for the relu chain at an iteration boundary
                    if prev_hj is not None:
                        d_proj(j - 1, prev_hj)
                    if j + 1 < JC:
                        nxt = build_xt(j + 1)
            prev_hj = hj


            # weave the serial GRU + small phase2 blocks into the PE stream
            if j >= 4 and j % 2 == 0 and (j - 4) // 2 < 8:
                gru_step((j - 4) // 2)
                if (j - 4) // 2 == 7:
                    btk_calc()
            if j in (20, 22, 24, 26):
                phase2_produce((j - 20) // 2)
            if j in (22, 24, 26, 28):
                phase2_mm((j - 22) // 2)

        d_proj(JC - 1, prev_hj)

        # phase2 log-softmax tail
        sg2 = fin.tile([4, D], F32, tag="sg2")
        nc.scalar.activation(sg2[:], sd2[:], AF.Sigmoid,
                             bias=wobbn[0:4, :], scale=-1.0)
        o2 = fin.tile([4, 2 * D], F32, tag="o2")
        o2e = _ap(o2[:], [o2[:].ap[0], [2, D]])
        nc.scalar.activation(o2e, sg2[:], AF.Ln)
        nc.vector.scalar_tensor_tensor(
            _ap(o2[:], [o2[:].ap[0], [2, D]], extra_offset=1),
            sd2[:], wobb[0:4, :], o2e, OP.add, OP.add)
        nc.sync.dma_start(dram["p2out"].ap(), o2[:])

        # phase3 log-softmax tail
        sg3 = fin.tile([JC, SC], F32, tag="sg3")
        nc.scalar.activation(sg3[:], sd3[:], AF.Sigmoid,
                             bias=clsbbn[0:JC, :], scale=-1.0)
        o3 = fin.tile([JC, 2 * SC], F32, tag="o3")
        o3e = _ap(o3[:], [o3[:].ap[0], [2, SC]])
        nc.scalar.activation(o3e, sg3[:], AF.Ln)
        nc.vector.scalar_tensor_tensor(
            _ap(o3[:], [o3[:].ap[0], [2, SC]], extra_offset=1),
            sd3[:], clsbb[0:JC, :], o3e, OP.add, OP.add)
        nc.sync.dma_start(dram["p3out"].ap(), o3[:])


# ------------------------------------------------------------------ host ----

def _chunk(w):
    """[512, N] -> [128, 4, N] flattened as [128, 4*N] (partition-chunked)."""
    n = w.shape[1]
    return np.ascontiguousarray(
        w.reshape(4, 128, n).transpose(1, 0, 2).reshape(128, 4 * n))


_CACHE = {}
TRACE = False          # test harness sets True to collect NTFF exec time


def kernel(text_emo, text_cau, label_ck, label3, chunksize, t_ratio,
           rep_w, rep_b, W_w, W_b, Wo_w, Wo_b,
           gru_wih, gru_whh, gru_bih, gru_bhh, W3_w, W3_b, cls_w, cls_b):
    text_emo = np.asarray(text_emo, np.float32)
    text_cau = np.asarray(text_cau, np.float32)
    label_ck = np.asarray(label_ck)
    label3 = np.asarray(label3)
    assert int(chunksize) == CS and int(t_ratio) == 1
    f32 = lambda x: np.asarray(x, np.float32)
    rep_w, rep_b, W_w, W_b, Wo_w, Wo_b = map(
        f32, (rep_w, rep_b, W_w, W_b, Wo_w, Wo_b))
    gru_wih, gru_whh, gru_bih, gru_bhh = map(
        f32, (gru_wih, gru_whh, gru_bih, gru_bhh))
    W3_w, W3_b, cls_w, cls_b = map(f32, (W3_w, W3_b, cls_w, cls_b))

    if "nc" not in _CACHE:
        _CACHE["nc"] = build_nc()
    nc = _CACHE["nc"]

    grub = np.concatenate([gru_bih[:2 * D] + gru_bhh[:2 * D],
                           gru_bih[2 * D:]])
    shared = {
        "tcauT": _chunk(text_cau[:, 0, :].T.copy()),
        "repT": _chunk(rep_w.T.copy()),
        "rep_b": np.ascontiguousarray(rep_b.reshape(4, 128).T),
        "w1T": _chunk(W_w[:, :D].T.copy()),
        "w2T": _chunk(W_w[:, D:].T.copy()),
        "wb_row": W_b.reshape(1, D).copy(),
        "wod": np.ascontiguousarray((Wo_w[1] - Wo_w[0]).reshape(4, 128).T),
        "wihT": _chunk(gru_wih.T.copy()),
        "whhT": _chunk(gru_whh.T.copy()),
        "grub_row": grub.reshape(1, 3 * D),
        "bhn_row": gru_bhh[2 * D:].reshape(1, D).copy(),
        "weT": _chunk(W3_w[:, :D].T.copy()),
        "wtT": _chunk(W3_w[:, D:2 * D].T.copy()),
        "wetT": _chunk(W3_w[:, 2 * D + 1:].T.copy()),
        "wn_row": W3_w[:, 2 * D].reshape(1, D).copy(),
        "b3_row": W3_b.reshape(1, D).copy(),
        "clsd": np.ascontiguousarray((cls_w[1] - cls_w[0]).reshape(4, 128).T),
        "ones_row": np.ones((1, 128), np.float32),
        "ones_blk": np.ones((128, JC), np.float32),
        "id128": np.eye(128, dtype=np.float32),
        "wobb": np.full((128, 1), Wo_b[1] - Wo_b[0], np.float32),
        "clsbb": np.full((128, 1), cls_b[1] - cls_b[0], np.float32),
        "wobbn": np.full((128, 1), Wo_b[0] - Wo_b[1], np.float32),
        "clsbbn": np.full((128, 1), cls_b[0] - cls_b[1], np.float32),
    }
    emoT = text_emo[:, 0, :].T.copy()      # [512, 256]
    in_maps = []
    for c in range(NCORES):
        m = dict(shared)
        m["temoT"] = _chunk(np.ascontiguousarray(
            emoT[:, c * JC:(c + 1) * JC]))
        in_maps.append(m)

    res = run_bass_kernel_spmd(nc, in_maps, core_ids=list(range(NCORES)),
                               trace=TRACE)
    _CACHE["last_res"] = res

    p2 = np.concatenate([res.results[c]["p2out"] for c in range(NCORES)], 0)
    p3d = np.concatenate(
        [res.results[c]["p3out"].reshape(JC, K, CS, 2)
         for c in range(NCORES)], 0)           # [256, 64, 8, 2]

    sel_j, sel_k = np.nonzero(label_ck)
    logit_out = np.ascontiguousarray(
        p3d[sel_j, sel_k].reshape(-1, 2), dtype=np.float32)

    l3 = label3.reshape(SE, -1)
    Lcau = np.zeros((SE, SC), np.float32)
    rows = np.repeat(np.arange(SE), l3.shape[1])
    cols = np.asarray(l3, np.int64).ravel()
    valid = cols >= 0
    Lcau[rows[valid], cols[valid]] = 1.0
    Lck = Lcau.reshape(SE, K, CS)
    label3_out = Lck[sel_j, sel_k].reshape(-1).astype(np.int32)

    return (np.asarray(p2, np.float32), logit_out, label3_out)
